# revision 9
# baseline (speedup 1.0000x reference)
"""EnhancedTernaryLinear on 8 Trainium2 NeuronCores.

out = (x @ W^T) * scale + bias
  x: [4, 2048, 4096] f32, W: [4096, 4096] ternary int8, scale/bias: [4096] f32

Strategy: data-parallel over tokens (8192 tokens -> 1024/core), W replicated.
Mixed-precision contraction: of the 32 k-subtiles (128 each), U_F8 are carried
as fp8-e4m3 and contracted with DoubleRow matmuls (2 subtiles per MM at ~1.9x
the bf16 row rate); the remaining C_BF subtiles are carried in bf16 (exact).
Ternary weights are exact in both fp8 and bf16, so all quantization error
comes from the fp8 x subtiles: rel_err ~= 0.0266 * sqrt(U_F8/32).

A host-side least-squares compensation folds a correction into the bf16
x values: delta = +pinv(WC^T) @ (WR^T eps) cancels the component of the fp8
quantization error that lies in the bf16 weight column space, scaling the
error by a further sqrt(1 - C_BF/32). With U_F8=22: 0.0266 * (22/32) ~= 1.8e-2.

Per core [O=4096, T=1024, K=4096]:
  - xr [P, U_F8, T] fp8 + xc [P, C_BF, T] bf16 resident in SBUF (host casts)
  - W streamed per 512-col o-chunk; one contiguous DMA per dtype per chunk
  - psum[o=128, t=512]: U_F8/2 DoubleRow MMs + C_BF bf16 MMs per tile
  - ScalarE: out = Identity(psum * scale[o] + bias[o]), bf16 out [O, T]
    (host upcasts to f32; costs ~0.1% quadrature error, halves store traffic)
"""

import numpy as np
import ml_dtypes

B, S, IN_F, OUT_F = 4, 2048, 4096, 4096
N_CORES = 8
TOKENS = B * S
T_PER_CORE = TOKENS // N_CORES

P = 128
KT = IN_F // P            # 32 k-subtiles
U_F8 = 22                 # fp8 subtiles (even)
C_BF = KT - U_F8          # bf16 subtiles
COMPENSATE = True         # host least-squares error compensation

F8 = ml_dtypes.float8_e4m3
BF16 = ml_dtypes.bfloat16


def _make_tile_context(nc):
    """TileContext whose end-of-kernel drain splits its sem waits.

    The stock ``_drain_and_barrier`` attaches one wait per logical proc to a
    single SP Drain; the walrus build in this container caps sync waits per
    instruction and rejects that ("Too many sync wait commands").  Emit the
    waits as individual EventSemaphore instructions instead (same semantics:
    SP blocks on each before joining the end-of-kernel barrier).
    """
    import concourse.mybir as mybir
    import concourse.tile as tile
    from concourse.vector_clock import ScopedClock

    class SplitDrainTileContext(tile.TileContext):
        def _commit_instruction(self, inst, lazy_reg_writes=True):
            si = inst.sync_info
            if si is not None and si.on_wait:
                cap = 2 if isinstance(inst, mybir.InstEventSemaphore) else 1
                waits = list(si.on_wait)
                if len(waits) > cap:
                    keep, excess = waits[:cap], waits[cap:]
                    for i in range(0, len(excess), 2):
                        chunk = excess[i:i + 2]
                        ev = mybir.InstEventSemaphore(
                            name=self.nc.get_next_instruction_name(),
                            ins=[],
                            outs=[],
                        )
                        ev.engine = inst.engine
                        ev.sync_info = mybir.SyncInfo(
                            on_wait=list(chunk), on_update=[]
                        )
                        super()._commit_instruction(ev)
                    si.on_wait.clear()
                    for w in keep:
                        si.on_wait.append(w)
            return super()._commit_instruction(inst, lazy_reg_writes)

        def _drain_and_barrier(self, tick_clock, wait_clock):
            nc = self.nc
            drain_inst = nc.sync.drain()
            wait_clock.add_sem_waits(
                drain_inst.ins, ScopedClock({None: tick_clock.global_clock})
            )
            si = drain_inst.ins.sync_info
            waits = list(si.on_wait) if si is not None and si.on_wait else []
            if len(waits) > 1:
                si.on_wait.clear()
                for i in range(0, len(waits), 2):
                    ev = mybir.InstEventSemaphore(
                        name=nc.get_next_instruction_name(), ins=[], outs=[]
                    )
                    ev.sync_info = mybir.SyncInfo(
                        on_wait=list(waits[i:i + 2]), on_update=[]
                    )
                    nc.sync.add_instruction(ev)

            nc.all_engine_barrier()
            assert self.sems is not None
            popped = nc._tile_sem_poison_stack.pop()
            assert popped is self._sem_poison
            nc.clear_and_free_semaphores(list(self.sems.allocated().values()))
            # no trailing all_engine_barrier: NEFF completion already waits
            # for every engine's stream end, and the sem clear is the last op
            # on its engine, so re-execution cannot observe stale sems.

    return SplitDrainTileContext(nc)


def _build(K, O, T, u=U_F8, c=C_BF):
    """Single-core Bass program: [O x T x K] GEMM, mixed fp8-DR/bf16 k-split."""
    import concourse.bass as bass
    import concourse.mybir as mybir

    NP = u // 2               # DoubleRow pair count
    NT = min(512, T)          # moving free dim per matmul
    TCH = T // NT             # t chunks
    OSUP_W = min(512, O)      # o columns per W staging load
    OSUP = O // OSUP_W
    OSUB = OSUP_W // P        # o tiles per W staging load
    OJ = O // P               # total o tiles

    DR = mybir.MatmulPerfMode.DoubleRow

    nc = bass.Bass()
    xr_d = nc.declare_dram_parameter("xr", [P, u, T], mybir.dt.float8e4, isOutput=False)
    xc_d = nc.declare_dram_parameter("xc", [P, c, T], mybir.dt.bfloat16, isOutput=False)
    wf_d = nc.declare_dram_parameter("wf", [P, OSUP, u, OSUP_W], mybir.dt.float8e4, isOutput=False)
    wb_d = nc.declare_dram_parameter("wb", [P, OSUP, c, OSUP_W], mybir.dt.bfloat16, isOutput=False)
    sc_d = nc.declare_dram_parameter("scale2", [P, OJ], mybir.dt.float32, isOutput=False)
    bi_d = nc.declare_dram_parameter("bias2", [P, OJ], mybir.dt.float32, isOutput=False)
    out_d = nc.declare_dram_parameter("out", [O, T], mybir.dt.bfloat16, isOutput=True)

    with _make_tile_context(nc) as tc:
        with (
            tc.tile_pool(name="consts", bufs=1) as consts,
            tc.tile_pool(name="xrp", bufs=NP) as xrp,
            tc.tile_pool(name="xcp", bufs=2) as xcp,
            tc.tile_pool(name="w0p", bufs=1) as w0p,
            tc.tile_pool(name="wrp", bufs=1) as wrp,
            tc.tile_pool(name="wb0p", bufs=1) as wb0p,
            tc.tile_pool(name="wf8", bufs=2) as wf8p,
            tc.tile_pool(name="wbf", bufs=2) as wbfp,
            tc.tile_pool(name="outp", bufs=8) as outp,
            tc.tile_pool(name="psum", bufs=8, space="PSUM") as psump,
        ):
            scale_sb = consts.tile([P, OJ], mybir.dt.float32)
            bias_sb = consts.tile([P, OJ], mybir.dt.float32)

            def drain_group(ps, j, tch):
                ot = outp.tile([P, NT], mybir.dt.bfloat16)
                nc.scalar.activation(
                    ot[:],
                    ps[:],
                    mybir.ActivationFunctionType.Identity,
                    bias=bias_sb[:, j:j + 1],
                    scale=scale_sb[:, j:j + 1],
                )
                # split out-stores across the two hwdge queues (ACT + SP):
                # halves the final store flush; W prefetch on SP has a full
                # osup of slack, so the head-of-line cost there is harmless.
                eng = nc.scalar if tch == 0 else nc.sync
                eng.dma_start(
                    out_d[j * P:(j + 1) * P, tch * NT:(tch + 1) * NT], ot[:]
                )

            # Startup: DIRECT2D issue costs ~0.6us per DMA on the SP queue, so
            # coalesce everything except the fp8 pairs the PE consumes first
            # (those stay granular so MM i only waits for pair i's transfer).
            # The first ~3.4us of real MMs run at the cold 1.2 GHz HAM rate;
            # a warmup block would cost more than it saves because it chains
            # behind another engine's queue bring-up.
            xr_ts = []      # per-pair [P, 2, T] fp8
            wf0 = None      # osup-0 fp8 W: pair 0 + rest chunk
            for i in range(NP):
                xt = xrp.tile([P, 2, T], mybir.dt.float8e4)
                nc.sync.dma_start(xt[:], xr_d[:, 2 * i:2 * i + 2, :])
                xr_ts.append(xt)
                if i == 0:
                    w0 = w0p.tile([P, 2, OSUP_W], mybir.dt.float8e4)
                    nc.sync.dma_start(w0[:], wf_d[:, 0, 0:2, :])
                if i == 1:
                    wr = wrp.tile([P, u - 2, OSUP_W], mybir.dt.float8e4)
                    nc.sync.dma_start(wr[:], wf_d[:, 0, 2:u, :])
                    wf0 = (w0, wr)
            # anchors: two coalesced x chunks + one W chunk
            c_lo = c // 2
            xc_a = xcp.tile([P, c_lo, T], mybir.dt.bfloat16)
            nc.sync.dma_start(xc_a[:], xc_d[:, 0:c_lo, :])
            xc_b = xcp.tile([P, c - c_lo, T], mybir.dt.bfloat16)
            nc.sync.dma_start(xc_b[:], xc_d[:, c_lo:c, :])
            wb0 = wb0p.tile([P, c, OSUP_W], mybir.dt.bfloat16)
            nc.sync.dma_start(wb0[:], wb_d[:, 0, :, :])

            def xc_slice(m, tch):
                src, off = (xc_a, 0) if m < c_lo else (xc_b, c_lo)
                return src[:, m - off, tch * NT:(tch + 1) * NT]

            def wf0_slice(i, osub):
                src, off = (wf0[0], 0) if i == 0 else (wf0[1], 1)
                return src[:, 2 * (i - off):2 * (i - off) + 2, osub * P:(osub + 1) * P]

            # scale/bias aren't needed until the first psum drain; keep them
            # out of the startup descriptor stream
            nc.sync.dma_start(scale_sb[:], sc_d[:])
            nc.sync.dma_start(bias_sb[:], bi_d[:])

            # osup 0: k-major per psum tile so the first MMs only depend on
            # the earliest DMAs in the startup stream.
            ps0 = [
                [
                    psump.tile([P, NT], mybir.dt.float32, tag="ps", name=f"ps0_{a}_{b}")
                    for b in range(TCH)
                ]
                for a in range(OSUB)
            ]
            for i in range(NP):
                for osub in range(OSUB):
                    for tch in range(TCH):
                        nc.tensor.matmul(
                            ps0[osub][tch][:],
                            wf0_slice(i, osub),
                            xr_ts[i][:, :, tch * NT:(tch + 1) * NT],
                            start=(i == 0),
                            stop=False,
                            perf_mode=DR,
                        )
            for m in range(c):
                for osub in range(OSUB):
                    for tch in range(TCH):
                        nc.tensor.matmul(
                            ps0[osub][tch][:],
                            wb0[:, m, osub * P:(osub + 1) * P],
                            xc_slice(m, tch),
                            start=False,
                            stop=(m == c - 1),
                        )
            for osub in range(OSUB):
                for tch in range(TCH):
                    drain_group(ps0[osub][tch], osub, tch)

            # osup 1..: x is resident; W arrives as one contiguous DMA per
            # dtype per osup (11 KB/partition runs), double-buffered.
            for osup in range(1, OSUP):
                wf_t = wf8p.tile([P, u, OSUP_W], mybir.dt.float8e4)
                nc.sync.dma_start(wf_t[:], wf_d[:, osup, :, :])
                wb_t = wbfp.tile([P, c, OSUP_W], mybir.dt.bfloat16)
                nc.sync.dma_start(wb_t[:], wb_d[:, osup, :, :])
                for osub in range(OSUB):
                    j = osup * OSUB + osub
                    for tch in range(TCH):
                        ps = psump.tile([P, NT], mybir.dt.float32, tag="ps")
                        for i in range(NP):
                            nc.tensor.matmul(
                                ps[:],
                                wf_t[:, 2 * i:2 * i + 2, osub * P:(osub + 1) * P],
                                xr_ts[i][:, :, tch * NT:(tch + 1) * NT],
                                start=(i == 0),
                                stop=False,
                                perf_mode=DR,
                            )
                        for m in range(c):
                            nc.tensor.matmul(
                                ps[:],
                                wb_t[:, m, osub * P:(osub + 1) * P],
                                xc_slice(m, tch),
                                start=False,
                                stop=(m == c - 1),
                            )
                        drain_group(ps, j, tch)
    return nc


_NC_CACHE = {}


def _get_nc():
    key = (IN_F, OUT_F, T_PER_CORE, U_F8)
    if key not in _NC_CACHE:
        _NC_CACHE[key] = _build(IN_F, OUT_F, T_PER_CORE)
    return _NC_CACHE[key]


def _prep_inputs(x, weight_ternary, weight_scale, bias):
    x = np.asarray(x)
    weight_ternary = np.asarray(weight_ternary)
    weight_scale = np.asarray(weight_scale)
    bias = np.asarray(bias)

    OSUP_W = 512
    OSUP = OUT_F // OSUP_W

    # x3[p, kt, t_all]
    x3 = np.ascontiguousarray(
        x.reshape(TOKENS, IN_F)
        .astype(np.float32, copy=False)
        .T.reshape(KT, P, TOKENS)
        .transpose(1, 0, 2)
    )
    xr_full = x3[:, :U_F8]                      # [P, u, T_all] f32
    xc_full = x3[:, U_F8:].astype(np.float32)   # [P, c, T_all] f32
    xr_q = xr_full.astype(F8)                   # fp8 payload

    # W columns, k-major per subtile: w3[p, kt, o]
    w3 = (
        weight_ternary.astype(np.float32)
        .T.reshape(KT, P, OUT_F)
        .transpose(1, 0, 2)
    )

    if COMPENSATE and C_BF > 0:
        # eps[k_raw, t]: fp8 quantization error of the raw subtiles
        eps = (xr_full - xr_q.astype(np.float32)).reshape(P * U_F8, TOKENS, order="F")
        WR = w3[:, :U_F8].reshape(P * U_F8, OUT_F, order="F")  # [2816, O]
        WC = w3[:, U_F8:].reshape(P * C_BF, OUT_F, order="F")  # [1280, O]
        # device computes WR^T (x_R - eps) + WC^T (x_C + delta); the in-space
        # part of the error -WR^T eps is cancelled by the lstsq delta
        G = WC @ WC.T                      # [1280, 1280]
        R = WC @ WR.T                      # [1280, 2816]
        Pmat = np.linalg.solve(G, R)       # [1280, 2816]
        delta = Pmat @ eps                 # [1280, T_all]
        xc_full = xc_full + delta.reshape(P, C_BF, TOKENS, order="F")

    xc_q = xc_full.astype(BF16)

    # W layouts: [P, OSUP, n, OSUP_W]
    wf = np.ascontiguousarray(
        w3[:, :U_F8].reshape(P, U_F8, OSUP, OSUP_W).transpose(0, 2, 1, 3)
    ).astype(F8)
    wb = np.ascontiguousarray(
        w3[:, U_F8:].reshape(P, C_BF, OSUP, OSUP_W).transpose(0, 2, 1, 3)
    ).astype(BF16)

    sc = np.ascontiguousarray(
        weight_scale.astype(np.float32, copy=False).reshape(OUT_F // P, P).T
    )  # [P, OJ]
    bi = np.ascontiguousarray(
        bias.astype(np.float32, copy=False).reshape(OUT_F // P, P).T
    )  # [P, OJ]

    in_maps = []
    for cid in range(N_CORES):
        tsl = slice(cid * T_PER_CORE, (cid + 1) * T_PER_CORE)
        in_maps.append(
            {
                "xr": np.ascontiguousarray(xr_q[:, :, tsl]),
                "xc": np.ascontiguousarray(xc_q[:, :, tsl]),
                "wf": wf,
                "wb": wb,
                "scale2": sc,
                "bias2": bi,
            }
        )
    return in_maps


def _assemble(results):
    # each core returns out [O, T_PER_CORE] bf16; upcast + transpose on host
    out = np.concatenate(
        [np.ascontiguousarray(r["out"].astype(np.float32).T) for r in results],
        axis=0,
    )  # [TOKENS, O]
    return out.reshape(B, S, OUT_F)


def _run(x, weight_ternary, weight_scale, bias, trace=False, **spmd_kwargs):
    import os
    import sys

    # the kernel needs the axon trn2 devices; guard against a harness that
    # pinned JAX_PLATFORMS=cpu (only effective before jax initializes)
    if "jax" not in sys.modules:
        plat = os.environ.get("JAX_PLATFORMS", "")
        if plat and "axon" not in plat:
            os.environ["JAX_PLATFORMS"] = "axon,cpu"

    from concourse.bass_utils import run_bass_kernel_spmd

    nc = _get_nc()
    in_maps = _prep_inputs(x, weight_ternary, weight_scale, bias)
    res = run_bass_kernel_spmd(
        nc, in_maps, core_ids=list(range(N_CORES)), trace=trace, **spmd_kwargs
    )
    return _assemble(res.results), res


def kernel(x, weight_ternary, weight_scale, bias):
    out, _ = _run(x, weight_ternary, weight_scale, bias, trace=False)
    return out
